# revision 1
# baseline (speedup 1.0000x reference)
"""Distributed Trainium2 kernel for nn_Attention (B=2, N=2048, C=1024, H=16, HD=64).

Sharding: sequence-parallel. Core c owns batch b=c//4 and query rows
[512*(c%4), 512*(c%4+1)).  Each core computes q/k/v for its own rows,
RoPEs q and k, AllGathers k^T and v (within its 4-core batch group),
then computes attention + projection for its row slice.  Outputs are
disjoint row slices of the final [B, N, C] tensor — no reduction needed.

All matmuls run in float32r (full-rate fp32).  Weights are pre-transposed
on the host so every matmul operand has its natural layout on device.
Attention is computed transposed (S^T = k^T q) so softmax denominators
come from an appended ones-column in v, and no on-device transposes are
ever needed.
"""

import sys

if "/opt/trn_rl_repo" not in sys.path:
    sys.path.insert(0, "/opt/trn_rl_repo")

import numpy as np

B, N, C = 2, 2048, 1024
H, HD = 16, 64
NCORES = 8
GB = 4          # cores per batch (replica group size)
NS = N // GB    # 512 rows per core
SC = HD ** -0.5  # attention scale


def build(mock_ag=False):
    import concourse.bass as bass
    import concourse.mybir as mybir
    import concourse.tile as tile
    from concourse import bacc

    f32 = mybir.dt.float32
    f32r = mybir.dt.float32r
    AF = mybir.ActivationFunctionType

    nc = bacc.Bacc(None, target_bir_lowering=False, num_devices=NCORES)

    # ---- per-core external inputs (host pre-shards / pre-transposes) ----
    xT = nc.declare_dram_parameter("xT", [C, NS], f32r, isOutput=False)
    wqkT = nc.declare_dram_parameter("wqkT", [C, 2 * C], f32r, isOutput=False)
    wvT = nc.declare_dram_parameter("wvT", [C, C], f32r, isOutput=False)
    wpT = nc.declare_dram_parameter("wpT", [C, C], f32r, isOutput=False)
    cos2 = nc.declare_dram_parameter("cos2", [128, NS], f32, isOutput=False)
    sins2 = nc.declare_dram_parameter("sins2", [128, NS], f32, isOutput=False)
    biasb = nc.declare_dram_parameter("biasb", [128, C], f32, isOutput=False)
    out = nc.declare_dram_parameter("out", [NS, C], f32, isOutput=True)

    groups = [list(range(GB)), list(range(GB, 2 * GB))]

    def mm(out_ap, lhsT_ap, rhs_ap, start, stop):
        nc.tensor.matmul(out_ap, lhsT_ap, rhs_ap, start=start, stop=stop)

    from contextlib import ExitStack

    with tile.TileContext(nc) as tc:
        with ExitStack() as stack:
            ep = stack.enter_context
            ep(nc.allow_low_precision(reason="f32r rounding of fp32 matmul inputs"))
            dramp = ep(tc.tile_pool(name="dram", bufs=1, space="DRAM"))
            constp = ep(tc.tile_pool(name="const", bufs=1))
            xtp = ep(tc.tile_pool(name="xTp", bufs=1))
            qtp = ep(tc.tile_pool(name="qTp", bufs=1))
            atp = ep(tc.tile_pool(name="aTp", bufs=1))
            wtsp = ep(tc.tile_pool(name="wts", bufs=20))
            ktmpp = ep(tc.tile_pool(name="ktmp", bufs=3))
            ropep = ep(tc.tile_pool(name="ropet", bufs=3))
            kheadp = ep(tc.tile_pool(name="khead", bufs=2))
            ptp = ep(tc.tile_pool(name="pTp", bufs=3))
            vhp_p = ep(tc.tile_pool(name="vhp", bufs=4))
            smallp = ep(tc.tile_pool(name="small", bufs=4))
            outp = ep(tc.tile_pool(name="outsb", bufs=3))
            ps_mm = ep(tc.tile_pool(name="ps_mm", bufs=2, space="PSUM"))
            ps_s = ep(tc.tile_pool(name="ps_s", bufs=2, space="PSUM"))
            ps_av = ep(tc.tile_pool(name="ps_av", bufs=2, space="PSUM"))

            # ---- internal DRAM for collectives (split by head half) ----
            k_inh, k_gathh, v_inh, v_gathh = [], [], [], []
            for s in range(2):
                k_inh.append(dramp.tile([C // 2, NS], f32r, name=f"k_in{s}"))
                k_gathh.append(
                    dramp.tile([GB, C // 2, NS], f32r, name=f"k_gath{s}")
                )
                v_inh.append(
                    dramp.tile([NS, 8, HD + 1], f32r, name=f"v_in{s}")
                )
                v_gathh.append(
                    dramp.tile([GB, NS, 8, HD + 1], f32r, name=f"v_gath{s}")
                )

            # ---- constants / persistent loads ----
            cos_sb = constp.tile([128, NS], f32, name="cos_sb")
            nc.sync.dma_start(cos_sb[:, :], cos2[:, :])
            sin_sb = constp.tile([128, NS], f32, name="sin_sb")
            nc.sync.dma_start(sin_sb[:, :], sins2[:, :])
            bias_sb = constp.tile([128, C], f32, name="bias_sb")
            nc.sync.dma_start(bias_sb[:, :], biasb[:, :])
            onesf = constp.tile([128, 64], f32, name="onesf")
            nc.vector.memset(onesf[:, :], 1.0)

            xT_sb = xtp.tile([128, 8, NS], f32r, name="xT_sb")
            for cc in range(8):
                nc.sync.dma_start(
                    xT_sb[:, cc, :], xT[cc * 128:(cc + 1) * 128, :]
                )

            qT_sb = qtp.tile([128, 8, NS], f32r, name="qT_sb")
            aT_sb = atp.tile([128, 8, NS], f32r, name="aT_sb")

            def rope_chunk(psum, dst):
                """dst = psum*cos + rot32(psum)*signed_sin, all [128, NS]."""
                tmp = ropep.tile([128, NS], f32, name="tmp", tag="ropetmp")
                for lo in (0, 64):
                    nc.vector.tensor_mul(
                        tmp[lo:lo + 32, :],
                        psum[lo + 32:lo + 64, :],
                        sin_sb[lo:lo + 32, :],
                    )
                    nc.vector.tensor_mul(
                        tmp[lo + 32:lo + 64, :],
                        psum[lo:lo + 32, :],
                        sin_sb[lo + 32:lo + 64, :],
                    )
                nc.vector.tensor_mul(dst, psum, cos_sb[:, :])
                nc.vector.tensor_add(dst, dst, tmp[:, :])

            # ---- v (natural [i, dv]) and k^T, in head halves; AG each ----
            def ag(in_t, out_t, tag):
                if mock_ag:
                    for r in range(GB):
                        nc.gpsimd.dma_start(out_t[r, 0:32], in_t[0:32])
                else:
                    nc.gpsimd.collective_compute(
                        "AllGather",
                        mybir.AluOpType.bypass,
                        replica_groups=groups,
                        ins=[in_t.opt()],
                        outs=[out_t.opt()],
                    )

            def kv_half(s):  # heads 8s..8s+7: v -> AG, k^T -> AG
                wv_tiles = []
                for cc in range(8):
                    w = wtsp.tile([128, 512], f32r, name="w", tag="wts")
                    nc.sync.dma_start(
                        w[:, :],
                        wvT[cc * 128:(cc + 1) * 128, s * 512:(s + 1) * 512],
                    )
                    wv_tiles.append(w)
                wk_tiles = []
                for cc in range(8):
                    w = wtsp.tile([128, 4, 128], f32r, name="w", tag="wts")
                    nc.scalar.dma_start(
                        w[:, :, :],
                        wqkT[
                            cc * 128:(cc + 1) * 128,
                            C + s * 512:C + (s + 1) * 512,
                        ].rearrange("p (m f) -> p m f", f=128),
                    )
                    wk_tiles.append(w)
                # v half
                for ic in range(4):
                    rows = slice(ic * 128, (ic + 1) * 128)
                    psum = ps_mm.tile([128, NS], f32, name="psum", tag="mm")
                    for cc in range(8):
                        mm(psum[:, :], xT_sb[:, cc, rows], wv_tiles[cc][:, :],
                           cc == 0, cc == 7)
                    vsb = outp.tile([128, 8, HD + 1], f32r, name="vsb", tag="osb")
                    nc.vector.tensor_copy(vsb[:, :, HD], onesf[:, 0:8])
                    nc.vector.tensor_copy(
                        vsb[:, :, 0:HD],
                        psum[:, :].rearrange("p (h d) -> p h d", d=HD),
                    )
                    nc.scalar.dma_start(v_inh[s][rows, :, :], vsb[:, :, :])
                ag(v_inh[s], v_gathh[s], f"v{s}")
                # k half
                for ml in range(4):
                    psum = ps_mm.tile([128, NS], f32, name="psum", tag="mm")
                    for cc in range(8):
                        mm(psum[:, :], wk_tiles[cc][:, ml, :], xT_sb[:, cc, :],
                           cc == 0, cc == 7)
                    kc = ktmpp.tile([128, NS], f32r, name="kc", tag="kc")
                    rope_chunk(psum[:, :], kc[:, :])
                    nc.scalar.dma_start(
                        k_inh[s][ml * 128:(ml + 1) * 128, :], kc[:, :]
                    )
                ag(k_inh[s], k_gathh[s], f"k{s}")

            def q_half(qh):  # q chunks 4qh..4qh+3 + rope
                wq_t = []
                for cc in range(8):
                    w = wtsp.tile([128, 4, 128], f32r, name="w", tag="wts")
                    nc.scalar.dma_start(
                        w[:, :, :],
                        wqkT[
                            cc * 128:(cc + 1) * 128, qh * 512:(qh + 1) * 512
                        ].rearrange("p (m f) -> p m f", f=128),
                    )
                    wq_t.append(w)
                for ml in range(4):
                    m = qh * 4 + ml
                    psum = ps_mm.tile([128, NS], f32, name="psum", tag="mm")
                    for cc in range(8):
                        mm(psum[:, :], wq_t[cc][:, ml, :],
                           xT_sb[:, cc, :], cc == 0, cc == 7)
                    rope_chunk(psum[:, :], qT_sb[:, m, :])

            kv_half(0)
            q_half(0)
            kv_half(1)
            q_half(1)

            # ---- attention, head pairs (flash-style over key chunks) ----
            vg = {}
            for hp in range(H // 2):  # heads 2*hp, 2*hp+1
                if hp % 4 == 0:  # prefetch v for heads [8*g, 8*(g+1))
                    g = hp // 4
                    for r in range(GB):
                        vt = vhp_p.tile(
                            [128, GB, 8, HD + 1], f32r, name="vt", tag="vt"
                        )
                        for half in range(2):
                            eng = [nc.gpsimd, nc.sync][(r + half) % 2]
                            eng.dma_start(
                                vt[:, half * 2:(half + 1) * 2, :, :],
                                v_gathh[g][
                                    r, half * 256:(half + 1) * 256, :, :
                                ].rearrange("(a p) h d -> p a h d", p=128),
                            )
                        vg[r] = vt
                kh = kheadp.tile([128, GB, NS], f32r, name="kh", tag="khead")
                kh_engines = [nc.gpsimd, nc.sync, nc.gpsimd, nc.sync]
                for r in range(GB):
                    kh_engines[r].dma_start(
                        kh[:, r, :],
                        k_gathh[hp // 4][
                            r, (hp % 4) * 128:(hp % 4 + 1) * 128, :
                        ],
                    )
                for sub in range(2):  # head h = 2*hp + sub at partitions sub*64
                    h = 2 * hp + sub
                    lo = sub * 64
                    q_ap = qT_sb[lo:lo + 64, hp, :]
                    po = ps_av.tile([HD + 1, NS], f32, name="po", tag="av")
                    for jp in range(8):  # pairs of key chunks
                        jc0 = 2 * jp
                        ps2 = ps_s.tile([128, 2, NS], f32, name="ps2", tag="sc")
                        for u in range(2):
                            jc = jc0 + u
                            r, jl = jc // 4, jc % 4
                            mm(ps2[:, u, :],
                               kh[lo:lo + 64, r, jl * 128:(jl + 1) * 128],
                               q_ap, True, True)
                        pt = ptp.tile([128, 2, NS], f32r, name="pt", tag="pT")
                        nc.scalar.activation(
                            pt[:, :, :], ps2[:, :, :], AF.Exp, scale=SC
                        )
                        for u in range(2):
                            jc = jc0 + u
                            r, jl = jc // 4, jc % 4
                            mm(po[:, :],
                               vg[r][:, jl, 2 * (hp % 4) + sub, :],
                               pt[:, u, :], jc == 0, jc == 15)
                    # normalize: reciprocal of denom row, gpsimd broadcast
                    recip = smallp.tile([1, NS], f32, name="recip", tag="recip")
                    nc.vector.reciprocal(recip[:, :], po[HD:HD + 1, :])
                    rb = smallp.tile([64, NS], f32, name="rb", tag="rb")
                    nc.gpsimd.partition_broadcast(rb[:, :], recip[:, :])
                    nc.vector.tensor_mul(
                        aT_sb[lo:lo + 64, hp, :], po[0:HD, :], rb[:, :]
                    )

            # ---- projection, two passes: pass 0 (heads 0-7) can run while
            # the second half of attention is still in flight ----
            wp_tiles = {}
            for nn in range(2):
                for cc in range(8):
                    w = wtsp.tile([128, 512], f32r, name="w", tag="wts")
                    nc.sync.dma_start(
                        w[:, :],
                        wpT[cc * 128:(cc + 1) * 128, nn * 512:(nn + 1) * 512],
                    )
                    wp_tiles[(nn, cc)] = w
            pacc = qtp.tile([128, 8, 512], f32, name="pacc")
            for ic in range(4):
                rows = slice(ic * 128, (ic + 1) * 128)
                for nn in range(2):
                    psum = ps_mm.tile([128, NS], f32, name="psum", tag="mm")
                    for cc in range(4):
                        mm(psum[:, :], aT_sb[:, cc, rows],
                           wp_tiles[(nn, cc)][:, :], cc == 0, cc == 3)
                    nc.vector.tensor_add(
                        pacc[:, ic * 2 + nn, :],
                        psum[:, :],
                        bias_sb[:, nn * 512:(nn + 1) * 512],
                    )
            for ic in range(4):
                rows = slice(ic * 128, (ic + 1) * 128)
                for nn in range(2):
                    psum = ps_mm.tile([128, NS], f32, name="psum", tag="mm")
                    for cc in range(4, 8):
                        mm(psum[:, :], aT_sb[:, cc, rows],
                           wp_tiles[(nn, cc)][:, :], cc == 4, cc == 7)
                    osb = outp.tile([128, 512], f32, name="osb", tag="osb")
                    nc.vector.tensor_add(
                        osb[:, :], psum[:, :], pacc[:, ic * 2 + nn, :]
                    )
                    nc.sync.dma_start(out[rows, nn * 512:(nn + 1) * 512], osb[:, :])

    nc.compile()
    return nc


_NC_CACHE = {}


def _get_nc():
    if "nc" not in _NC_CACHE:
        _NC_CACHE["nc"] = build()
    return _NC_CACHE["nc"]


def make_in_maps(x, cos, sin, qkv_w, proj_w, proj_b):
    x = np.asarray(x, np.float32)
    cos = np.asarray(cos, np.float32)
    sin = np.asarray(sin, np.float32)
    qkv_w = np.asarray(qkv_w, np.float32)
    proj_w = np.asarray(proj_w, np.float32)
    proj_b = np.asarray(proj_b, np.float32)

    wqkT = np.ascontiguousarray(qkv_w[: 2 * C].T)        # [C, 2C]
    wvT = np.ascontiguousarray(qkv_w[2 * C:].T)          # [C, C]
    wpT = np.ascontiguousarray(proj_w.T)                 # [C, C]
    biasb = np.ascontiguousarray(np.broadcast_to(proj_b, (128, C)))
    sign = np.concatenate([-np.ones(32, np.float32), np.ones(32, np.float32)])

    in_maps = []
    for c in range(NCORES):
        b, r = c // GB, c % GB
        rows = slice(r * NS, (r + 1) * NS)
        xTc = np.ascontiguousarray(x[b, rows].T)          # [C, NS]
        cosT = cos[rows].T                                # [HD, NS]
        sinsT = (sin[rows] * sign).T                      # [HD, NS] signed
        cos2v = np.ascontiguousarray(np.concatenate([cosT, cosT], 0))   # [128, NS]
        sins2v = np.ascontiguousarray(np.concatenate([sinsT, sinsT], 0))
        in_maps.append(
            {
                "xT": xTc,
                "wqkT": wqkT,
                "wvT": wvT,
                "wpT": wpT,
                "cos2": cos2v,
                "sins2": sins2v,
                "biasb": biasb,
            }
        )
    return in_maps


def assemble(results):
    out = np.empty((B, N, C), np.float32)
    for c in range(NCORES):
        b, r = c // GB, c % GB
        out[b, r * NS:(r + 1) * NS] = results[c]["out"]
    return out


def kernel(x, cos, sin, qkv_w, proj_w, proj_b):
    from concourse.bass_utils import run_bass_kernel_spmd

    nc = _get_nc()
    in_maps = make_in_maps(x, cos, sin, qkv_w, proj_w, proj_b)
    res = run_bass_kernel_spmd(nc, in_maps, core_ids=list(range(NCORES)))
    return assemble(res.results)



# revision 6
# speedup vs baseline: 2.4254x; 2.4254x over previous
"""Distributed Trainium2 kernel for nn_Attention (B=2, N=2048, C=1024, H=16, HD=64).

Sharding: tensor-parallel over heads.  Core c owns batch b=c//4 and heads
[4*(c%4), 4*(c%4)+4) over the FULL sequence.  Each core computes q/k/v for
its heads (RoPE on q,k), dense softmax attention, and its partial
projection; partials are summed with four pipelined bf16 ReduceScatters
(one per 512-row block) so each core ends with disjoint row slices of the
final output.  No AllGathers are needed at all.

Attention is computed with scores transposed (s^T = k^T q -> [keys, q]) so
exp output feeds the AV matmul as lhsT directly; AV is non-transposed
(out [q, hd+1]) with a ones-column in v producing softmax denominators.
The attention output is transposed for the projection with the DMA xbar
(nearly free).  exp runs on the ACT engine (the critical path), k-RoPE on
Pool, q-RoPE + normalize + PSUM->SBUF copies on DVE.
"""

import sys

if "/opt/trn_rl_repo" not in sys.path:
    sys.path.insert(0, "/opt/trn_rl_repo")

import numpy as np

B, N, C = 2, 2048, 1024
H, HD = 16, 64
NCORES = 8
GB = 4            # tensor-parallel group size (cores per batch)
HPC = H // GB     # 4 heads per core
SC = HD ** -0.5   # attention scale
NQC = N // 512    # 4 query chunks of 512
NKC = N // 128    # 16 key chunks of 128


def build():
    import concourse.bass as bass
    import concourse.mybir as mybir
    import concourse.tile as tile
    from concourse import bacc
    from contextlib import ExitStack

    f32 = mybir.dt.float32
    f32r = mybir.dt.float32r
    bf16 = mybir.dt.bfloat16
    AF = mybir.ActivationFunctionType

    nc = bacc.Bacc(None, target_bir_lowering=False, num_devices=NCORES)

    # ---- per-core external inputs (host pre-shards / pre-transposes) ----
    xT = nc.declare_dram_parameter("xT", [C, N], bf16, isOutput=False)
    wqT = nc.declare_dram_parameter("wqT", [C, 256], bf16, isOutput=False)
    wkT = nc.declare_dram_parameter("wkT", [C, 256], bf16, isOutput=False)
    wvT = nc.declare_dram_parameter("wvT", [C, 256], bf16, isOutput=False)
    wpT = nc.declare_dram_parameter("wpT", [256, C], bf16, isOutput=False)
    cos2 = nc.declare_dram_parameter("cos2", [128, N], f32, isOutput=False)
    sins2 = nc.declare_dram_parameter("sins2", [128, N], f32, isOutput=False)
    out = nc.declare_dram_parameter("out", [512, C], bf16, isOutput=True)

    groups = [list(range(GB)), list(range(GB, 2 * GB))]
    mm = nc.tensor.matmul

    with tile.TileContext(nc) as tc:
        with ExitStack() as stack:
            ep = stack.enter_context
            ep(nc.allow_low_precision(reason="bf16 attention within 2e-2 gate"))
            dramp = ep(tc.tile_pool(name="dram", bufs=1, space="DRAM"))
            constp = ep(tc.tile_pool(name="const", bufs=1))
            xtp = ep(tc.tile_pool(name="xTp", bufs=1))
            wp_ = ep(tc.tile_pool(name="wts", bufs=1))
            qkp = ep(tc.tile_pool(name="qk", bufs=1))
            vp = ep(tc.tile_pool(name="vsb", bufs=1))
            ptp = ep(tc.tile_pool(name="pT", bufs=16))
            ropep = ep(tc.tile_pool(name="ropet", bufs=4))
            attnp = ep(tc.tile_pool(name="attn", bufs=1))
            attnTp = ep(tc.tile_pool(name="attnT", bufs=1))
            outp = ep(tc.tile_pool(name="outsb", bufs=3))
            smallp = ep(tc.tile_pool(name="small", bufs=8))
            ps_mm = ep(tc.tile_pool(name="ps_mm", bufs=2, space="PSUM"))
            ps_s = ep(tc.tile_pool(name="ps_s", bufs=2, space="PSUM"))
            ps_av = ep(tc.tile_pool(name="ps_av", bufs=2, space="PSUM"))

            # separate DRAM staging tile per RS block: avoids false
            # (tensor-granularity) deps between RS_j reads and proj_{j+1} writes
            rs_in = [
                dramp.tile([512, C], bf16, name=f"rs_in{j}") for j in range(4)
            ]

            # ---- persistent SBUF tiles ----
            # weights + cos/sin via swdge (Pool queue) so hwdge queues stay
            # free for xT and the ACT queue issues no DMAs before exp
            wk_sb = wp_.tile([128, 8, 256], bf16, name="wk_sb")
            wq_sb = wp_.tile([128, 8, 256], bf16, name="wq_sb")
            for cc in range(8):
                nc.gpsimd.dma_start(wk_sb[:, cc, :], wkT[cc * 128:(cc + 1) * 128, :])
            for cc in range(8):
                nc.gpsimd.dma_start(wq_sb[:, cc, :], wqT[cc * 128:(cc + 1) * 128, :])
            cos_sb = constp.tile([128, N], f32, name="cos_sb")
            sin_sb = constp.tile([128, N], f32, name="sin_sb")
            nc.gpsimd.dma_start(cos_sb[:, :], cos2[:, :])
            nc.gpsimd.dma_start(sin_sb[:, :], sins2[:, :])

            xT_sb = xtp.tile([128, 8, N], bf16, name="xT_sb")
            for nc_ in range(NQC):
                for cc in range(8):
                    nc.sync.dma_start(
                        xT_sb[:, cc, nc_ * 512:(nc_ + 1) * 512],
                        xT[cc * 128:(cc + 1) * 128, nc_ * 512:(nc_ + 1) * 512],
                    )
            wv_sb = wp_.tile([128, 8, 256], bf16, name="wv_sb")
            for cc in range(8):
                nc.sync.dma_start(wv_sb[:, cc, :], wvT[cc * 128:(cc + 1) * 128, :])
            wp_sb = wp_.tile([128, 2, C], bf16, name="wp_sb")
            for ch in range(2):
                nc.sync.dma_start(wp_sb[:, ch, :], wpT[ch * 128:(ch + 1) * 128, :])

            qT_sb = qkp.tile([128, 2, N], f32r, name="qT_sb")
            kT_sb = qkp.tile([128, 2, N], f32r, name="kT_sb")
            v_sb = vp.tile([128, NKC, HPC, HD + 1], bf16, name="v_sb")
            # softmax-denominator ones column, set once
            nc.gpsimd.memset(v_sb[:, :, :, HD:HD + 1], 1.0)

            attn_sb = attnp.tile([128, 16, HPC, HD], bf16, name="attn_sb")
            attnT = [
                attnTp.tile([128, N], bf16, name=f"attnT{chh}") for chh in range(2)
            ]

            def rope_chunk(eng, psum, dst, n0):
                """dst = psum*cos + rot32(psum)*signed_sin, all [128, 512]."""
                tmp = ropep.tile([128, 512], f32, name="tmp", tag="ropetmp")
                for lo in (0, 64):
                    eng.tensor_mul(
                        tmp[lo:lo + 32, :], psum[lo + 32:lo + 64, :],
                        sin_sb[lo:lo + 32, n0:n0 + 512],
                    )
                    eng.tensor_mul(
                        tmp[lo + 32:lo + 64, :], psum[lo:lo + 32, :],
                        sin_sb[lo + 32:lo + 64, n0:n0 + 512],
                    )
                eng.tensor_mul(dst, psum, cos_sb[:, n0:n0 + 512])
                eng.tensor_add(dst, dst, tmp[:, :])

            def emit_kq(hp):
                # k for head pair hp: psum [128 (2h x 64d), 512 n] -> rope(Pool)
                for nc_ in range(NQC):
                    ps = ps_mm.tile([128, 512], f32, name="ps", tag="mm")
                    for cc in range(8):
                        mm(ps[:, :], wk_sb[:, cc, hp * 128:(hp + 1) * 128],
                           xT_sb[:, cc, nc_ * 512:(nc_ + 1) * 512],
                           start=cc == 0, stop=cc == 7)
                    rope_chunk(nc.gpsimd, ps[:, :],
                               kT_sb[:, hp, nc_ * 512:(nc_ + 1) * 512], nc_ * 512)
                for nc_ in range(NQC):
                    ps = ps_mm.tile([128, 512], f32, name="ps", tag="mm")
                    for cc in range(8):
                        mm(ps[:, :], wq_sb[:, cc, hp * 128:(hp + 1) * 128],
                           xT_sb[:, cc, nc_ * 512:(nc_ + 1) * 512],
                           start=cc == 0, stop=cc == 7)
                    rope_chunk(nc.vector, ps[:, :],
                               qT_sb[:, hp, nc_ * 512:(nc_ + 1) * 512], nc_ * 512)

            def emit_v():
                # v[keys, dv] in bf16 with ones column; copies on DVE
                for kc in range(NKC):
                    ps = ps_mm.tile([128, 512], f32, name="ps", tag="mm")
                    for cc in range(8):
                        mm(ps[:, 0:256], xT_sb[:, cc, kc * 128:(kc + 1) * 128],
                           wv_sb[:, cc, :], start=cc == 0, stop=cc == 7)
                    nc.vector.tensor_copy(
                        v_sb[:, kc, :, 0:HD],
                        ps[:, 0:256].rearrange("p (h d) -> p h d", d=HD),
                    )

            pt_tiles = {}

            def emit_scores(hp, qc):
                # s^T = k^T q per head of the pair; exp on ACT -> pT bf16
                for sub in range(2):
                    h = 2 * hp + sub
                    lo = sub * 64
                    q_ap = qT_sb[lo:lo + 64, hp, qc * 512:(qc + 1) * 512]
                    for g in range(4):
                        pt = ptp.tile([128, 4, 512], bf16, name="pt", tag="pt")
                        pt_tiles[(h, qc, g)] = pt
                        for pr in range(2):
                            ps = ps_s.tile([128, 2, 512], f32, name="ps2", tag="sc")
                            for u in range(2):
                                kc = 4 * g + 2 * pr + u
                                mm(ps[:, u, :],
                                   kT_sb[lo:lo + 64, hp, kc * 128:(kc + 1) * 128],
                                   q_ap, start=True, stop=True)
                            nc.scalar.activation(
                                pt[:, 2 * pr:2 * pr + 2, :], ps[:, :, :],
                                AF.Exp, scale=SC,
                            )

            def emit_av(hp, qc):
                # out[q, hd+1] per head; lhsT = p^T chunks, rhs = v (+ones)
                for sub in range(2):
                    h = 2 * hp + sub
                    av = ps_av.tile([128, 4, HD + 1], f32, name="av", tag="av")
                    pts = [pt_tiles.pop((h, qc, g)) for g in range(4)]
                    for qs in range(4):
                        for g in range(4):
                            for u in range(4):
                                kc = 4 * g + u
                                mm(av[:, qs, :],
                                   pts[g][:, u, qs * 128:(qs + 1) * 128],
                                   v_sb[:, kc, h, :],
                                   start=kc == 0, stop=kc == NKC - 1)
                    for qs in range(4):
                        qb = qc * 4 + qs
                        recip = smallp.tile([128, 1], f32, name="recip", tag="rc")
                        nc.vector.reciprocal(recip[:, :], av[:, qs, HD:HD + 1])
                        nc.vector.tensor_scalar_mul(
                            attn_sb[:, qb, h, :], av[:, qs, 0:HD], recip[:, :]
                        )

            def emit_proj(qc):
                for qs in range(4):
                    qb = qc * 4 + qs
                    for chh in range(2):
                        nc.sync.dma_start_transpose(
                            attnT[chh][:, qb * 128:(qb + 1) * 128],
                            attn_sb[:, qb, 2 * chh:2 * chh + 2, :],
                        )
                for qs in range(4):
                    qb = qc * 4 + qs
                    osb = outp.tile([128, C], bf16, name="osb", tag="osb")
                    for nn in range(2):
                        ps = ps_mm.tile([128, 512], f32, name="ps", tag="mm")
                        for chh in range(2):
                            mm(ps[:, :], attnT[chh][:, qb * 128:(qb + 1) * 128],
                               wp_sb[:, chh, nn * 512:(nn + 1) * 512],
                               start=chh == 0, stop=chh == 1)
                        nc.vector.tensor_copy(osb[:, nn * 512:(nn + 1) * 512],
                                              ps[:, :])
                    nc.sync.dma_start(
                        rs_in[qc][qs * 128:(qs + 1) * 128, :], osb[:, :]
                    )

            def emit_rs(j):
                nc.gpsimd.collective_compute(
                    "ReduceScatter", mybir.AluOpType.add,
                    replica_groups=groups,
                    ins=[rs_in[j][:, :].opt()],
                    outs=[out[j * 128:(j + 1) * 128, :].opt()],
                )

            # ---- emission order (per-engine program order drives overlap) ----
            emit_kq(0)
            emit_scores(0, 0)
            emit_kq(1)
            emit_scores(1, 0)
            emit_v()
            emit_av(0, 0)
            emit_scores(0, 1)
            emit_av(1, 0)
            emit_scores(1, 1)
            emit_proj(0)
            emit_rs(0)
            emit_av(0, 1)
            emit_scores(0, 2)
            emit_av(1, 1)
            emit_scores(1, 2)
            emit_proj(1)
            emit_rs(1)
            emit_av(0, 2)
            emit_scores(0, 3)
            emit_av(1, 2)
            emit_scores(1, 3)
            emit_proj(2)
            emit_rs(2)
            emit_av(0, 3)
            emit_av(1, 3)
            emit_proj(3)
            emit_rs(3)

    nc.compile()
    return nc


_NC_CACHE = {}


def _get_nc():
    if "nc" not in _NC_CACHE:
        _NC_CACHE["nc"] = build()
    return _NC_CACHE["nc"]


def make_in_maps(x, cos, sin, qkv_w, proj_w, proj_b):
    import ml_dtypes

    bf16 = ml_dtypes.bfloat16
    x = np.asarray(x, np.float32)
    cos = np.asarray(cos, np.float32)
    sin = np.asarray(sin, np.float32)
    qkv_w = np.asarray(qkv_w, np.float32)
    proj_w = np.asarray(proj_w, np.float32)

    sign = np.concatenate([-np.ones(32, np.float32), np.ones(32, np.float32)])
    cosT = cos.T                                       # [HD, N]
    sinsT = (sin * sign).T                             # [HD, N] signed
    cos2v = np.ascontiguousarray(np.concatenate([cosT, cosT], 0))   # [128, N]
    sins2v = np.ascontiguousarray(np.concatenate([sinsT, sinsT], 0))

    in_maps = []
    for c in range(NCORES):
        b, hg = c // GB, c % GB
        cs = slice(256 * hg, 256 * hg + 256)
        in_maps.append(
            {
                "xT": np.ascontiguousarray(x[b].T).astype(bf16),
                "wqT": np.ascontiguousarray(qkv_w[0 * C:1 * C][cs].T).astype(bf16),
                "wkT": np.ascontiguousarray(qkv_w[1 * C:2 * C][cs].T).astype(bf16),
                "wvT": np.ascontiguousarray(qkv_w[2 * C:3 * C][cs].T).astype(bf16),
                "wpT": np.ascontiguousarray(proj_w[:, cs].T).astype(bf16),
                "cos2": cos2v,
                "sins2": sins2v,
            }
        )
    return in_maps


def assemble(results, proj_b):
    out = np.empty((B, N, C), np.float32)
    for c in range(NCORES):
        b, r = c // GB, c % GB
        o = np.asarray(results[c]["out"]).astype(np.float32)
        for j in range(4):
            out[b, 512 * j + 128 * r: 512 * j + 128 * r + 128] = (
                o[128 * j:128 * (j + 1)]
            )
    return out + np.asarray(proj_b, np.float32)


def kernel(x, cos, sin, qkv_w, proj_w, proj_b):
    from concourse.bass_utils import run_bass_kernel_spmd

    nc = _get_nc()
    in_maps = make_in_maps(x, cos, sin, qkv_w, proj_w, proj_b)
    res = run_bass_kernel_spmd(nc, in_maps, core_ids=list(range(NCORES)))
    return assemble(res.results, proj_b)


# revision 16
# speedup vs baseline: 2.8796x; 1.1872x over previous
"""Distributed Trainium2 kernel for nn_Attention (B=2, N=2048, C=1024, H=16, HD=64).

Sharding: tensor-parallel over heads.  Core c owns batch b=c//4 and heads
[4*(c%4), 4*(c%4)+4) over the FULL sequence.  Each core computes q/k/v for
its heads (RoPE on q,k), dense softmax attention, and its partial
projection; partials are summed with four pipelined bf16 ReduceScatters
(one per 512-row block) so each core ends with disjoint row slices of the
final output.  No AllGathers are needed at all.

Attention is computed with scores transposed (s^T = k^T q -> [keys, q]) so
exp output feeds the AV matmul as lhsT directly; AV is non-transposed
(out [q, hd+1]) with a ones-column in v producing softmax denominators.
The attention output is transposed for the projection on the PE (identity
matmul; the DMA-xbar transpose would serialize with collectives).  exp
runs on the ACT engine (the critical path, ~133us busy); k-RoPE on Pool,
q-RoPE + normalize + PSUM->SBUF copies on DVE.

Emission is generator-based: each phase's score/exp chunks are woven with
AV chunks of earlier phases (and V / second-head-pair QKV / projection
chunks) so the ACT engine never starves while PE stays in-order.
"""

import sys

if "/opt/trn_rl_repo" not in sys.path:
    sys.path.insert(0, "/opt/trn_rl_repo")

import numpy as np

B, N, C = 2, 2048, 1024
H, HD = 16, 64
NCORES = 8
GB = 4            # tensor-parallel group size (cores per batch)
HPC = H // GB     # 4 heads per core
SC = HD ** -0.5   # attention scale
NQC = N // 512    # 4 query chunks of 512
NKC = N // 128    # 16 key chunks of 128


def build():
    import concourse.bass as bass
    import concourse.mybir as mybir
    import concourse.tile as tile
    from concourse import bacc
    from contextlib import ExitStack

    f32 = mybir.dt.float32
    f32r = mybir.dt.float32r
    bf16 = mybir.dt.bfloat16
    AF = mybir.ActivationFunctionType

    nc = bacc.Bacc(None, target_bir_lowering=False, num_devices=NCORES)

    # ---- per-core external inputs (host pre-shards / pre-transposes) ----
    xT = nc.declare_dram_parameter("xT", [C, N], bf16, isOutput=False)
    wqT = nc.declare_dram_parameter("wqT", [C, 256], bf16, isOutput=False)
    wkT = nc.declare_dram_parameter("wkT", [C, 256], bf16, isOutput=False)
    wvT = nc.declare_dram_parameter("wvT", [C, 256], bf16, isOutput=False)
    wpT = nc.declare_dram_parameter("wpT", [256, C], bf16, isOutput=False)
    cos2 = nc.declare_dram_parameter("cos2", [128, N], bf16, isOutput=False)
    sins2 = nc.declare_dram_parameter("sins2", [128, N], bf16, isOutput=False)
    out = nc.declare_dram_parameter("out", [512, C], bf16, isOutput=True)

    groups = [list(range(GB)), list(range(GB, 2 * GB))]
    mm = nc.tensor.matmul

    with tile.TileContext(nc) as tc:
        with ExitStack() as stack:
            ep = stack.enter_context
            ep(nc.allow_low_precision(reason="bf16 attention within 2e-2 gate"))
            dramp = ep(tc.tile_pool(name="dram", bufs=1, space="DRAM"))
            constp = ep(tc.tile_pool(name="const", bufs=1))
            xtp = ep(tc.tile_pool(name="xTp", bufs=1))
            wp_ = ep(tc.tile_pool(name="wts", bufs=1))
            qkp = ep(tc.tile_pool(name="qk", bufs=1))
            vp = ep(tc.tile_pool(name="vsb", bufs=1))
            ptp = ep(tc.tile_pool(name="pT", bufs=18))
            ropep = ep(tc.tile_pool(name="ropet", bufs=4))
            attnp = ep(tc.tile_pool(name="attn", bufs=1))
            attnTp = ep(tc.tile_pool(name="attnT", bufs=1))
            outp = ep(tc.tile_pool(name="outsb", bufs=3))
            smallp = ep(tc.tile_pool(name="small", bufs=8))
            ps_mm = ep(tc.tile_pool(name="ps_mm", bufs=2, space="PSUM"))
            ps_s = ep(tc.tile_pool(name="ps_s", bufs=2, space="PSUM"))
            ps_av = ep(tc.tile_pool(name="ps_av", bufs=2, space="PSUM"))

            # separate DRAM staging tile per RS block: avoids false
            # (tensor-granularity) deps between RS_j reads and proj_{j+1} writes
            rs_in = [
                dramp.tile([512, C], bf16, name=f"rs_in{j}") for j in range(4)
            ]

            # ---- persistent SBUF tiles ----
            # weights + cos/sin via swdge (Pool queue) so hwdge queues stay
            # free for xT and the ACT queue issues no DMAs before exp.
            cos_sb = constp.tile([128, N], bf16, name="cos_sb")
            sin_sb = constp.tile([128, N], bf16, name="sin_sb")
            nc.gpsimd.dma_start(cos_sb[:, :], cos2[:, :])
            nc.gpsimd.dma_start(sin_sb[:, :], sins2[:, :])
            wk_sb = wp_.tile([128, 8, 256], bf16, name="wk_sb")
            wq_sb = wp_.tile([128, 8, 256], bf16, name="wq_sb")
            for cc in range(8):
                nc.gpsimd.dma_start(wk_sb[:, cc, :], wkT[cc * 128:(cc + 1) * 128, :])
            for cc in range(8):
                nc.gpsimd.dma_start(wq_sb[:, cc, :], wqT[cc * 128:(cc + 1) * 128, :])

            # identity (bf16) for PE transposes; DMA-xbar transposes would
            # serialize with the collectives, so transpose on PE instead
            id_sb = constp.tile([128, 128], bf16, name="id_sb")
            nc.gpsimd.memset(id_sb[:, :], 1.0)
            nc.gpsimd.affine_select(
                id_sb[:, :], id_sb[:, :], pattern=[[1, 128]],
                compare_op=mybir.AluOpType.is_equal, fill=0.0,
                base=0, channel_multiplier=-1,
            )

            xT_sb = xtp.tile([128, 8, N], bf16, name="xT_sb")
            for nc_ in range(NQC):
                for cc in range(8):
                    nc.sync.dma_start(
                        xT_sb[:, cc, nc_ * 512:(nc_ + 1) * 512],
                        xT[cc * 128:(cc + 1) * 128, nc_ * 512:(nc_ + 1) * 512],
                    )
            wv_sb = wp_.tile([128, 8, 256], bf16, name="wv_sb")
            for cc in range(8):
                nc.sync.dma_start(wv_sb[:, cc, :], wvT[cc * 128:(cc + 1) * 128, :])
            wp_sb = wp_.tile([128, 2, C], bf16, name="wp_sb")
            for ch in range(2):
                nc.sync.dma_start(wp_sb[:, ch, :], wpT[ch * 128:(ch + 1) * 128, :])

            qT_sb = qkp.tile([128, 2, N], f32r, name="qT_sb")
            kT_sb = qkp.tile([128, 2, N], f32r, name="kT_sb")
            v_sb = vp.tile([128, NKC, HPC, HD + 1], bf16, name="v_sb")
            # softmax-denominator ones column, set once
            nc.gpsimd.memset(v_sb[:, :, :, HD:HD + 1], 1.0)

            attn_sb = attnp.tile([128, 16, HPC, HD], bf16, name="attn_sb")
            attnT_sb = attnTp.tile([128, 2, N], bf16, name="attnT_sb")

            def rope_chunk(eng, psum, dst, n0):
                """dst = psum*cos + rot32(psum)*signed_sin, all [128, 512]."""
                tmp = ropep.tile([128, 512], f32, name="tmp", tag="ropetmp")
                for lo in (0, 64):
                    eng.tensor_mul(
                        tmp[lo:lo + 32, :], psum[lo + 32:lo + 64, :],
                        sin_sb[lo:lo + 32, n0:n0 + 512],
                    )
                    eng.tensor_mul(
                        tmp[lo + 32:lo + 64, :], psum[lo:lo + 32, :],
                        sin_sb[lo + 32:lo + 64, n0:n0 + 512],
                    )
                eng.tensor_mul(dst, psum, cos_sb[:, n0:n0 + 512])
                eng.tensor_add(dst, dst, tmp[:, :])

            def kq_gen(hp):
                # 8 chunks: k (4 n-chunks, rope on Pool) then q (rope on DVE)
                for which, wsb, dstT, reng in (
                    ("k", wk_sb, kT_sb, nc.gpsimd),
                    ("q", wq_sb, qT_sb, nc.vector),
                ):
                    for nc_ in range(NQC):
                        ps = ps_mm.tile([128, 512], f32, name="ps", tag="mm")
                        for cc in range(8):
                            mm(ps[:, :], wsb[:, cc, hp * 128:(hp + 1) * 128],
                               xT_sb[:, cc, nc_ * 512:(nc_ + 1) * 512],
                               start=cc == 0, stop=cc == 7)
                        rope_chunk(reng, ps[:, :],
                                   dstT[:, hp, nc_ * 512:(nc_ + 1) * 512],
                                   nc_ * 512)
                        yield

            def v_gen():
                # 16 chunks: v[keys, dv] in bf16 (+ones col); copies on DVE
                for kc in range(NKC):
                    ps = ps_mm.tile([128, 512], f32, name="ps", tag="mm")
                    for cc in range(8):
                        mm(ps[:, 0:256], xT_sb[:, cc, kc * 128:(kc + 1) * 128],
                           wv_sb[:, cc, :], start=cc == 0, stop=cc == 7)
                    nc.vector.tensor_copy(
                        v_sb[:, kc, :, 0:HD],
                        ps[:, 0:256].rearrange("p (h d) -> p h d", d=HD),
                    )
                    yield

            pt_tiles = {}

            def scores_gen(hp, qc):
                # 8 chunks (sub, g): 4 score mms + 2 exp instrs -> pT bf16
                for sub in range(2):
                    h = 2 * hp + sub
                    lo = sub * 64
                    q_ap = qT_sb[lo:lo + 64, hp, qc * 512:(qc + 1) * 512]
                    for g in range(4):
                        pt = ptp.tile([128, 4, 512], bf16, name="pt", tag="pt")
                        pt_tiles[(h, qc, g)] = pt
                        for pr in range(2):
                            ps = ps_s.tile([128, 2, 512], f32, name="ps2", tag="sc")
                            for u in range(2):
                                kc = 4 * g + 2 * pr + u
                                mm(ps[:, u, :],
                                   kT_sb[lo:lo + 64, hp, kc * 128:(kc + 1) * 128],
                                   q_ap, start=True, stop=True)
                            nc.scalar.activation(
                                pt[:, 2 * pr:2 * pr + 2, :], ps[:, :, :],
                                AF.Exp, scale=SC,
                            )
                        yield

            def av_gen(hp, qc):
                # 16 chunks; one accumulation group open per PSUM tile
                for sub in range(2):
                    h = 2 * hp + sub
                    pts = [pt_tiles[(h, qc, g)] for g in range(4)]
                    for qp in range(2):
                        av = ps_av.tile([128, 2, HD + 1], f32, name="av",
                                        tag="av")
                        for i in range(2):
                            qs = qp * 2 + i
                            for gp in range(2):
                                for g in (2 * gp, 2 * gp + 1):
                                    for u in range(4):
                                        kc = 4 * g + u
                                        mm(av[:, i, :],
                                           pts[g][:, u,
                                                  qs * 128:(qs + 1) * 128],
                                           v_sb[:, kc, h, :],
                                           start=kc == 0, stop=kc == NKC - 1)
                                yield
                        for i in range(2):
                            qs = qp * 2 + i
                            qb = qc * 4 + qs
                            recip = smallp.tile([128, 1], f32, name="recip",
                                                tag="rc")
                            nc.vector.reciprocal(recip[:, :],
                                                 av[:, i, HD:HD + 1])
                            nc.vector.tensor_scalar_mul(
                                attn_sb[:, qb, h, :], av[:, i, 0:HD],
                                recip[:, :]
                            )
                    if sub == 1:
                        for g in range(4):
                            del pt_tiles[(h, qc, g)]

            def proj_gen(qc):
                # 4 chunks (per qb): PE transpose + proj matmuls + copy + DMA
                for qs in range(4):
                    qb = qc * 4 + qs
                    # [128, 2, 130] bf16 = 520B matches the av tag byte size,
                    # so transposes share the av PSUM slots
                    tr = ps_av.tile([128, 2, 130], bf16, name="tr", tag="av")
                    for chh in range(2):
                        mm(tr[:, chh, 0:128],
                           attn_sb[:, qb, 2 * chh:2 * chh + 2, :],
                           id_sb[:, :], is_transpose=True,
                           start=True, stop=True)
                    nc.vector.tensor_copy(
                        attnT_sb[:, :, qb * 128:(qb + 1) * 128], tr[:, :, 0:128]
                    )
                    osb = outp.tile([128, C], bf16, name="osb", tag="osb")
                    for nn in range(2):
                        ps = ps_mm.tile([128, 512], f32, name="ps", tag="mm")
                        for chh in range(2):
                            mm(ps[:, :],
                               attnT_sb[:, chh, qb * 128:(qb + 1) * 128],
                               wp_sb[:, chh, nn * 512:(nn + 1) * 512],
                               start=chh == 0, stop=chh == 1)
                        nc.vector.tensor_copy(osb[:, nn * 512:(nn + 1) * 512],
                                              ps[:, :])
                    nc.sync.dma_start(rs_in[qc][qs * 128:(qs + 1) * 128, :],
                                      osb[:, :])
                    yield

            def emit_rs(j):
                nc.gpsimd.collective_compute(
                    "ReduceScatter", mybir.AluOpType.add,
                    replica_groups=groups,
                    ins=[rs_in[j][:, :].opt()],
                    outs=[out[j * 128:(j + 1) * 128, :].opt()],
                )

            def weave(slots, *pairs):
                for _ in range(slots):
                    for gen, cnt in pairs:
                        for _ in range(cnt):
                            next(gen, None)

            def drain(gen):
                for _ in gen:
                    pass

            # ---- schedule: phases p0..p7 = s(0,0) s(1,0) s(0,1) s(1,1)
            #      s(0,2) s(1,2) s(0,3) s(1,3); AV of phase k weaves under
            #      phase k+2; V under p1; kq(1) under p0; proj(qc)+RS under
            #      the phase after av(1,qc) completes ----
            for _ in kq_gen(0):
                pass
            g = {}
            g["s00"], g["kq1"] = scores_gen(0, 0), kq_gen(1)
            weave(8, (g["s00"], 1), (g["kq1"], 1))
            g["s10"], g["v"] = scores_gen(1, 0), v_gen()
            weave(8, (g["s10"], 1), (g["v"], 2))
            g["s01"], g["a00"] = scores_gen(0, 1), av_gen(0, 0)
            weave(8, (g["s01"], 1), (g["a00"], 2))
            g["s11"], g["a10"], g["a01"] = (
                scores_gen(1, 1), av_gen(1, 0), av_gen(0, 1))
            weave(8, (g["s11"], 1), (g["a10"], 2), (g["a01"], 2))
            g["s02"], g["a11"], g["p0"] = (
                scores_gen(0, 2), av_gen(1, 1), proj_gen(0))
            weave(4, (g["s02"], 1), (g["a11"], 4))
            weave(4, (g["s02"], 1), (g["p0"], 1))
            emit_rs(0)
            g["s12"], g["a02"], g["p1"] = (
                scores_gen(1, 2), av_gen(0, 2), proj_gen(1))
            weave(4, (g["s12"], 1), (g["a02"], 4))
            weave(4, (g["s12"], 1), (g["p1"], 1))
            emit_rs(1)
            g["s03"], g["a12"], g["p2"] = (
                scores_gen(0, 3), av_gen(1, 2), proj_gen(2))
            weave(4, (g["s03"], 1), (g["a12"], 4))
            weave(4, (g["s03"], 1), (g["p2"], 1))
            emit_rs(2)
            g["s13"], g["a03"] = scores_gen(1, 3), av_gen(0, 3)
            weave(8, (g["s13"], 1), (g["a03"], 2))
            drain(av_gen(1, 3))
            drain(proj_gen(3))
            emit_rs(3)

    nc.compile()
    return nc


_NC_CACHE = {}


def _get_nc():
    if "nc" not in _NC_CACHE:
        _NC_CACHE["nc"] = build()
    return _NC_CACHE["nc"]


def make_in_maps(x, cos, sin, qkv_w, proj_w, proj_b):
    import ml_dtypes

    bf16 = ml_dtypes.bfloat16
    x = np.asarray(x, np.float32)
    cos = np.asarray(cos, np.float32)
    sin = np.asarray(sin, np.float32)
    qkv_w = np.asarray(qkv_w, np.float32)
    proj_w = np.asarray(proj_w, np.float32)

    sign = np.concatenate([-np.ones(32, np.float32), np.ones(32, np.float32)])
    cosT = cos.T                                       # [HD, N]
    sinsT = (sin * sign).T                             # [HD, N] signed
    cos2v = np.ascontiguousarray(np.concatenate([cosT, cosT], 0)).astype(bf16)
    sins2v = np.ascontiguousarray(np.concatenate([sinsT, sinsT], 0)).astype(bf16)

    in_maps = []
    for c in range(NCORES):
        b, hg = c // GB, c % GB
        cs = slice(256 * hg, 256 * hg + 256)
        in_maps.append(
            {
                "xT": np.ascontiguousarray(x[b].T).astype(bf16),
                "wqT": np.ascontiguousarray(qkv_w[0 * C:1 * C][cs].T).astype(bf16),
                "wkT": np.ascontiguousarray(qkv_w[1 * C:2 * C][cs].T).astype(bf16),
                "wvT": np.ascontiguousarray(qkv_w[2 * C:3 * C][cs].T).astype(bf16),
                "wpT": np.ascontiguousarray(proj_w[:, cs].T).astype(bf16),
                "cos2": cos2v,
                "sins2": sins2v,
            }
        )
    return in_maps


def assemble(results, proj_b):
    out = np.empty((B, N, C), np.float32)
    for c in range(NCORES):
        b, r = c // GB, c % GB
        o = np.asarray(results[c]["out"]).astype(np.float32)
        for j in range(4):
            out[b, 512 * j + 128 * r: 512 * j + 128 * r + 128] = (
                o[128 * j:128 * (j + 1)]
            )
    return out + np.asarray(proj_b, np.float32)


def kernel(x, cos, sin, qkv_w, proj_w, proj_b):
    from concourse.bass_utils import run_bass_kernel_spmd

    nc = _get_nc()
    in_maps = make_in_maps(x, cos, sin, qkv_w, proj_w, proj_b)
    res = run_bass_kernel_spmd(nc, in_maps, core_ids=list(range(NCORES)))
    return assemble(res.results, proj_b)
